# revision 1
# baseline (speedup 1.0000x reference)
"""Trainium2 Bass kernel for nn_BatchFlipLoss (NCE batch-flip loss + CE loss).

Math reformulation (validated to rel-err ~1e-7 vs the jax reference):

The reference sums BatchCriterion over 36 flip-class pairs (i,j), j>=i.
For pair (i,j) with x = [f_i; f_j] (f_c = features[c::8], L2-normalized,
B=512 rows each), T=0.1, the loss decomposes over ordered halves (a,b)
(rows of f_a, partner f_b). With E_ab = exp(10*G_ab), G_ab = f_a@f_b.T,
S_ab = rowsum(E_ab), S2_ab = rowsum(E_ab^2), d_ab[p] = f_a[p].f_b[p]:

  D_ab  = S0_aa + S_ab      (S0_aa: diag-zeroed; (a,a): D = 2*S0_aa+e^10)
  half  = 10*d - ln(D) - [N1*R + N2*R^2/2] - ln(1 - exp(10 d)*R)
          where R = 1/D, N_k = Sk0_aa + Sk_ab   (ln(1-x) ~ -(x+x^2/2);
          the x^3 tail is ~1e-6 relative after the alpha/1024 scaling)
  (a,a) pair = 2*(10*d - lnD - [N1*R + N2*R^2/2]), N_k = 2*Sk0_aa
          (the cross-diag term cancels -ln(1-pmt) exactly)

Work sharing: only the 36 unordered blocks are exponentiated. Core c
computes blocks (c, c+j mod 8) for j=0..4 (every unordered pair {a,b}
appears: distance k<=4 on core a, else distance 8-k on core b; the four
distance-4 pairs are computed twice, once per endpoint). Each block
yields BOTH directions' sums:
  rowsum  S_{c,c+j}   : ScalarE Exp accum_out / VectorE stt accum_out
  colsum  S_{c+j,c}   : PE matmul with a ones lhsT, accumulated over the
                        four row-chunks in a [1,512] PSUM bank (only
                        needed for j in {1,2,3}; distance-4 halves use
                        each endpoint's own rowsum)
The host reroutes these O(rows) vectors between cores and applies the
closed-form combine; CE rowsums (exp-accum + iota/is_equal gather) ride
along. All O(N^2) work (matmuls, exp, squares) stays on device.

SPMD: one NEFF for all cores, written for canonical class 0; the host
rotates each core's feature inputs so its own class is block 0 and the
partners are blocks 1..4.
"""

from contextlib import ExitStack

import numpy as np

FLIP = 8
B = 512
D = 128
C = 400
N = 4096
ALPHA = 0.03
E10 = float(np.exp(np.float32(10.0)))
NJ = 5  # partner blocks per core (distances 0..4)

_CACHE = {}


def _build_nc(ebufs=8, sbufs=8, pbufs=4, gbufs=4, cbufs=2):
    import concourse.tile as tile
    from concourse import bacc, mybir

    f32 = mybir.dt.float32
    bf16 = mybir.dt.bfloat16
    AF = mybir.ActivationFunctionType
    OP = mybir.AluOpType
    AX = mybir.AxisListType

    nc = bacc.Bacc("TRN2", target_bir_lowering=False, debug=False)

    ft_d = nc.dram_tensor("ft", [D, NJ * B], bf16, kind="ExternalInput")
    fr_d = nc.dram_tensor("fr", [4, 128, NJ, D], bf16, kind="ExternalInput")
    pred_d = nc.dram_tensor("pred", [B, C], f32, kind="ExternalInput")
    lab_d = nc.dram_tensor("lab", [B, 1], f32, kind="ExternalInput")
    iota_d = nc.dram_tensor("iota", [128, C], f32, kind="ExternalInput")
    eye_d = nc.dram_tensor("eye40", [128, 128], f32, kind="ExternalInput")
    m1_d = nc.dram_tensor("m1", [128, 20], f32, kind="ExternalOutput")
    m2_d = nc.dram_tensor("m2", [128, 20], f32, kind="ExternalOutput")
    dt_d = nc.dram_tensor("dt", [128, 20], f32, kind="ExternalOutput")
    cs1_d = nc.dram_tensor("cs1", [NJ, B], f32, kind="ExternalOutput")
    cs2_d = nc.dram_tensor("cs2", [NJ, B], f32, kind="ExternalOutput")
    ce_d = nc.dram_tensor("ce", [128, 8], f32, kind="ExternalOutput")

    with tile.TileContext(nc) as tc, ExitStack() as ctx:
        const = ctx.enter_context(tc.tile_pool(name="const", bufs=1))
        gpool = ctx.enter_context(tc.tile_pool(name="gp", bufs=gbufs, space="PSUM"))
        cpool = ctx.enter_context(tc.tile_pool(name="cp", bufs=cbufs, space="PSUM"))
        epool = ctx.enter_context(tc.tile_pool(name="ep", bufs=ebufs))
        spool = ctx.enter_context(tc.tile_pool(name="sp", bufs=sbufs))
        fpool = ctx.enter_context(tc.tile_pool(name="fp", bufs=2))
        ppool = ctx.enter_context(tc.tile_pool(name="pp", bufs=pbufs))
        small = ctx.enter_context(tc.tile_pool(name="sm", bufs=1))

        ftt = const.tile([D, NJ * B], bf16)
        iott = const.tile([128, C], f32)
        eyet = const.tile([128, 128], f32)
        ones = const.tile([128, 1], bf16)
        nc.vector.memset(ones[:], 1.0)
        M1 = small.tile([128, 20], f32)
        M2 = small.tile([128, 20], f32)
        dte = small.tile([128, 20], f32)
        cet = small.tile([128, 8], f32)

        # All input DMAs up front, hand-ordered: first ft block 0 (unblocks
        # the Gram pipeline), then the CE inputs, then the rest.
        pred4 = const.tile([128, 4, C], f32)
        lab4 = const.tile([128, 4], f32)
        fr_all = const.tile([128, 4, NJ, D], bf16)
        nc.sync.dma_start(ftt[:, 0:B], ft_d[:, 0:B])
        nc.sync.dma_start(lab4[:], lab_d[:, :].rearrange("(c p) k -> p (c k)", p=128))
        nc.sync.dma_start(iott[:], iota_d[:, :])
        nc.sync.dma_start(pred4[:], pred_d[:, :].rearrange("(c p) k -> p c k", p=128))
        nc.sync.dma_start(eyet[:], eye_d[:, :])
        for j in range(1, NJ):
            nc.sync.dma_start(ftt[:, j * B : (j + 1) * B], ft_d[:, j * B : (j + 1) * B])
        nc.sync.dma_start(fr_all[:], fr_d[:, :, :, :].rearrange("r p j k -> p r j k"))

        # ---- CE loss rowsums (this core's 512 rows of predicts) ----
        for c in range(4):
            mask = ppool.tile([128, C], f32)
            nc.vector.tensor_scalar(
                mask[:], iott[:], lab4[:, c : c + 1], None, OP.is_equal
            )
            scr = ppool.tile([128, C], f32)
            nc.vector.scalar_tensor_tensor(
                scr[:], mask[:], 1.0, pred4[:, c, :], OP.mult, OP.mult,
                accum_out=cet[:, 4 + c : 5 + c],
            )
            scr2 = ppool.tile([128, C], f32)
            nc.scalar.activation(
                scr2[:], pred4[:, c, :], AF.Exp, bias=0.0, scale=1.0,
                accum_out=cet[:, c : c + 1],
            )

        # ---- d_ab[p] = f_a[row] . f_b[row], all partners at once ----
        dvw = dte[:].rearrange("p (j r) -> p j r", r=4)
        for r in range(4):
            frt = fr_all[:, r, :, :]
            prod = fpool.tile([128, NJ, D], f32)
            nc.gpsimd.tensor_tensor(
                prod[:], frt, frt[:, 0:1, :].to_broadcast([128, NJ, D]), OP.mult
            )
            nc.vector.tensor_reduce(
                dvw[:, :, r], prod[:], axis=AX.X, op=OP.add
            )

        # ---- Gram blocks + moments (j outer so colsum PSUM accums are
        #      only live within one j iteration) ----
        for j in range(NJ):
            need_cs = j in (1, 2, 3)
            if need_cs:
                cs1t = cpool.tile([1, B], f32, tag="cs1t")
                cs2t = cpool.tile([1, B], f32, tag="cs2t")
            for r in range(4):
                cidx = j * 4 + r
                gt = gpool.tile([128, B], f32)
                nc.tensor.matmul(
                    gt[:],
                    ftt[:, r * 128 : (r + 1) * 128],
                    ftt[:, j * B : (j + 1) * B],
                    start=True,
                    stop=True,
                )
                if j == 0:
                    # own-block diag: g_pp(=1) -> g_pp-40 so exp(k*10*g)=0
                    nc.vector.tensor_sub(
                        gt[:, r * 128 : (r + 1) * 128],
                        gt[:, r * 128 : (r + 1) * 128],
                        eyet[:],
                    )
                et = epool.tile([128, B], bf16)
                nc.scalar.activation(
                    et[:], gt[:], AF.Exp, bias=0.0, scale=10.0,
                    accum_out=M1[:, cidx : cidx + 1],
                )
                e2 = spool.tile([128, B], bf16)
                nc.vector.scalar_tensor_tensor(
                    e2[:], et[:], 1.0, et[:], OP.mult, OP.mult,
                    accum_out=M2[:, cidx : cidx + 1],
                )
                if need_cs:
                    nc.tensor.matmul(
                        cs1t[:], ones[:], et[:],
                        start=(r == 0), stop=(r == 3),
                    )
                    nc.tensor.matmul(
                        cs2t[:], ones[:], e2[:],
                        start=(r == 0), stop=(r == 3),
                    )
            if need_cs:
                # PSUM can't DMA directly; stage via SBUF (ACT/DVE split)
                cs1s = spool.tile([1, B], f32, tag="cs1s")
                nc.scalar.copy(cs1s[:], cs1t[:])
                nc.sync.dma_start(cs1_d[j : j + 1, :], cs1s[:])
                cs2s = spool.tile([1, B], f32, tag="cs2s")
                nc.vector.tensor_copy(cs2s[:], cs2t[:])
                nc.sync.dma_start(cs2_d[j : j + 1, :], cs2s[:])

        nc.sync.dma_start(m1_d[:, :], M1[:])
        nc.sync.dma_start(m2_d[:, :], M2[:])
        nc.sync.dma_start(dt_d[:, :], dte[:])
        nc.sync.dma_start(ce_d[:, :], cet[:])

    nc.compile()
    return nc


def _get_nc(**kw):
    key = tuple(sorted(kw.items()))
    if key not in _CACHE:
        _CACHE[key] = _build_nc(**kw)
    return _CACHE[key]


def _prep_in_maps(predicts, labels, features):
    import ml_dtypes

    feats = np.ascontiguousarray(features, dtype=np.float32)
    pred = np.ascontiguousarray(predicts, dtype=np.float32)
    lab = np.asarray(labels).astype(np.float32).reshape(N, 1)
    f8 = feats.reshape(B, FLIP, D).transpose(1, 0, 2)  # [8,512,128], f8[c]=feats[c::8]
    iota = np.ascontiguousarray(
        np.broadcast_to(np.arange(C, dtype=np.float32), (128, C))
    )
    eye40 = (40.0 * np.eye(128)).astype(np.float32)
    in_maps = []
    for a in range(FLIP):
        order = [(a + i) % FLIP for i in range(NJ)]
        fo = f8[order]  # [5, 512, 128]: own class then distance 1..4 partners
        ft = np.ascontiguousarray(fo.transpose(2, 0, 1).reshape(D, NJ * B)).astype(
            ml_dtypes.bfloat16
        )
        fr = np.ascontiguousarray(
            fo.transpose(1, 0, 2).reshape(4, 128, NJ, D)
        ).astype(ml_dtypes.bfloat16)
        in_maps.append(
            {
                "ft": ft,
                "fr": fr,
                "pred": np.ascontiguousarray(pred[a * B : (a + 1) * B]),
                "lab": np.ascontiguousarray(lab[a * B : (a + 1) * B]),
                "iota": iota,
                "eye40": eye40,
            }
        )
    return in_maps


def _combine(outs):
    """Host-side O(rows) combine: reroute per-block sums between the
    ordered halves, apply the closed-form series, sum the partials."""
    S1 = {}
    S2 = {}
    dv = {}
    for c in range(FLIP):
        m1 = np.asarray(outs[c]["m1"], np.float64)  # [128, 20] cols j*4+r
        m2 = np.asarray(outs[c]["m2"], np.float64)
        dt = np.asarray(outs[c]["dt"], np.float64)
        cs1 = np.asarray(outs[c]["cs1"], np.float64)  # [5, 512], rows 1..3 used
        cs2 = np.asarray(outs[c]["cs2"], np.float64)
        for j in range(NJ):
            b = (c + j) % FLIP
            # rowsum vectors over rows of f_c: chunk r -> rows 128r..128r+127
            S1[(c, b)] = m1[:, j * 4 : (j + 1) * 4].T.reshape(B)
            S2[(c, b)] = m2[:, j * 4 : (j + 1) * 4].T.reshape(B)
            dv[(c, b)] = dt[:, j * 4 : (j + 1) * 4].T.reshape(B)
            dv[(b, c)] = dv[(c, b)]  # d is batch-indexed, symmetric in (a,b)
        for j in (1, 2, 3):
            # colsum of block (c, c+j) = rowsum of block (c+j, c)
            S1[((c + j) % FLIP, c)] = cs1[j]
            S2[((c + j) % FLIP, c)] = cs2[j]

    nce = 0.0
    for a in range(FLIP):
        S10 = S1[(a, a)]  # diag-zeroed own-block rowsum
        S20 = S2[(a, a)]
        for b in range(FLIP):
            d = dv[(a, b)]
            if a == b:
                N1 = 2.0 * S10
                N2 = 2.0 * S20
                Dv = N1 + E10
                R = 1.0 / Dv
                half = 10.0 * d - np.log(Dv) - (N1 * R + 0.5 * N2 * R * R)
                nce += 2.0 * half.sum()
            else:
                N1 = S10 + S1[(a, b)]
                N2 = S20 + S2[(a, b)]
                Dv = N1
                R = 1.0 / Dv
                half = (
                    10.0 * d
                    - np.log(Dv)
                    - (N1 * R + 0.5 * N2 * R * R)
                    - np.log1p(-np.exp(10.0 * d) * R)
                )
                nce += half.sum()

    ce = 0.0
    for c in range(FLIP):
        se = np.asarray(outs[c]["ce"], np.float64)[:, 0:4]
        xlab = np.asarray(outs[c]["ce"], np.float64)[:, 4:8]
        ce += (np.log(se) - xlab).sum()
    val = ALPHA * (-(nce) / 1024.0) + ce / N
    return np.array(val, dtype=np.float32)


def _run_hw(in_maps, trace=False):
    from concourse.bass_utils import run_bass_kernel_spmd

    nc = _get_nc()
    res = run_bass_kernel_spmd(nc, in_maps, core_ids=list(range(FLIP)), trace=trace)
    return res


def kernel(predicts, labels, features, indexs=None, **_):
    in_maps = _prep_in_maps(predicts, labels, features)
    res = _run_hw(in_maps)
    return _combine(res.results)


def kernel_sim(predicts, labels, features, indexs=None, **_):
    """CoreSim (CPU simulator) path for fast correctness iteration."""
    from concourse.bass_interp import CoreSim

    nc = _get_nc()
    in_maps = _prep_in_maps(predicts, labels, features)
    outs = []
    for a in range(FLIP):
        sim = CoreSim(nc, trace=False)
        for k, v in in_maps[a].items():
            sim.tensor(k)[:] = v
        sim.simulate()
        outs.append(
            {
                k: np.array(sim.tensor(k))
                for k in ("m1", "m2", "dt", "cs1", "cs2", "ce")
            }
        )
    return _combine(outs)



# revision 7
# speedup vs baseline: 1.6303x; 1.6303x over previous
"""Trainium2 Bass kernel for nn_BatchFlipLoss (NCE batch-flip loss + CE loss).

Math reformulation (validated ~1e-7 vs the jax reference in f64; the
first-order series below adds ~9e-5, vs a 2e-2 gate):

The reference sums BatchCriterion over 36 flip-class pairs (i,j), j>=i.
For pair (i,j) with x = [f_i; f_j] (f_c = features[c::8], L2-normalized,
B=512 rows each), T=0.1, the loss decomposes over ordered halves (a,b).
With E_ab = exp(10*G_ab), G_ab = f_a@f_b.T, S_ab = rowsum(E_ab),
d_ab[p] = f_a[p].f_b[p]:

  D_ab = S0_aa + S_ab      (S0_aa: diag-removed; (a,a): D = 2*S0_aa+e^10)
  half = 10*d - ln(D) - N1/D - ln(1 - exp(10 d)/D),  N1 = S0_aa + S_ab
  (a,a) pair = 2*(10*d - lnD - 2*S0_aa/D)
  ln(1-x) ~ -x only (the x^2/2 tail is ~9e-5 relative after scaling).

Device work per core c (SPMD, inputs rotated so its class is block 0):
  - 20 Gram tiles [128,512] = blocks (c, c+j), j=0..4, four 128-row
    chunks each.  PE matmuls write groups of 3 tiles into one 3-bank
    PSUM tile; ONE ACT exp [128,1536] converts each group to bf16 E.
  - per-tile rowsums: DVE tensor_scalar(+accum_out) on the bf16 E slice
    (2x DVE mode; accum is free) -> out[:, t].
  - colsums for j in {1,2,3} (the partner core's rowsums): PE matmuls
    with one-hot lhsT columns accumulate all three j's into a single
    [3,512] PSUM tile; staged to SBUF by Pool, one DMA.
  - CE: one ACT exp over [128,1600] predicts, Pool tensor_scalar(+accum)
    per 400-class chunk -> out[:, 20+c].
  - diag of block (c,c) is NOT zeroed on device: the host subtracts
    bf16(exp(10*||f_p||^2_bf16)) from the raw diag rowsum (error from
    the activation-LUT mismatch is O(1e-3) relative at worst).
The host does only O(N*D)/O(N) work: input layout, d_ab = f_a[p].f_b[p]
products, the CE label gather, and the closed-form scalar combine.
"""

from contextlib import ExitStack

import numpy as np

FLIP = 8
B = 512
D = 128
C = 400
N = 4096
ALPHA = 0.03
E10 = float(np.exp(np.float32(10.0)))
NJ = 5        # partner blocks per core (distances 0..4)
NT = 4 * NJ   # 20 Gram tiles of [128, 512]
GS = 3        # tiles per PSUM exp group

_CACHE = {}


def _groups():
    """Tile ids grouped GS at a time: [[0,1,2],[3,4,5],...,[18,19]]."""
    return [list(range(g, min(g + GS, NT))) for g in range(0, NT, GS)]


def _build_nc():
    import concourse.tile as tile
    from concourse import bacc, mybir

    f32 = mybir.dt.float32
    bf16 = mybir.dt.bfloat16
    AF = mybir.ActivationFunctionType
    OP = mybir.AluOpType

    nc = bacc.Bacc("TRN2", target_bir_lowering=False, debug=False)

    FT_COLS = NJ * B + 16          # 5 blocks + one-hot cols (9 used, pad 16)
    ft_d = nc.dram_tensor("ft", [D, FT_COLS], bf16, kind="ExternalInput")
    pred_d = nc.dram_tensor("pred", [128, 4 * C], f32, kind="ExternalInput")
    out_d = nc.dram_tensor("out", [128, NT + 4], f32, kind="ExternalOutput")
    cs1_d = nc.dram_tensor("cs1", [3, B], f32, kind="ExternalOutput")

    groups = _groups()
    csmm_tiles = [t for t in range(NT) if 1 <= t // 4 <= 3]

    with tile.TileContext(nc) as tc, ExitStack() as ctx:
        const = ctx.enter_context(tc.tile_pool(name="const", bufs=1))
        gpool = ctx.enter_context(tc.tile_pool(name="gp", bufs=2, space="PSUM"))
        cpool = ctx.enter_context(tc.tile_pool(name="cp", bufs=1, space="PSUM"))
        epool = ctx.enter_context(tc.tile_pool(name="ep", bufs=3))
        small = ctx.enter_context(tc.tile_pool(name="sm", bufs=1))

        ftt = const.tile([D, FT_COLS], bf16)
        predt = const.tile([128, 4 * C], f32)
        outt = small.tile([128, NT + 4], f32)
        scr = small.tile([128, GS * B], bf16)   # dummy main-out for rowsums
        scrp = small.tile([128, C], bf16)       # dummy main-out for CE sums
        cs1s = small.tile([3, B], f32)

        nc.sync.dma_start(ftt[:, 0:B], ft_d[:, 0:B])
        nc.sync.dma_start(ftt[:, B:FT_COLS], ft_d[:, B:FT_COLS])
        nc.sync.dma_start(predt[:], pred_d[:, :])

        cs1p = cpool.tile([3, B], f32)

        pg = {}
        eg = {}

        def emit_mms(g):
            tiles = groups[g]
            pgt = gpool.tile([128, len(tiles) * B], f32, tag="pg")
            pg[g] = pgt
            for s, t in enumerate(tiles):
                j, r = t // 4, t % 4
                nc.tensor.matmul(
                    pgt[:, s * B : (s + 1) * B],
                    ftt[:, r * 128 : (r + 1) * 128],
                    ftt[:, j * B : (j + 1) * B],
                    start=True,
                    stop=True,
                )

        def emit_cs(g):
            for s, t in enumerate(groups[g]):
                j = t // 4
                if not (1 <= j <= 3):
                    continue
                oh = NJ * B + 3 * (j - 1)
                nc.tensor.matmul(
                    cs1p[:],
                    ftt[:, oh : oh + 3],
                    eg[g][:, s * B : (s + 1) * B],
                    start=(t == csmm_tiles[0]),
                    stop=(t == csmm_tiles[-1]),
                )

        def emit_exp(g):
            tiles = groups[g]
            egt = epool.tile([128, len(tiles) * B], bf16, tag="eg")
            eg[g] = egt
            nc.scalar.activation(egt[:], pg[g][:], AF.Exp, bias=0.0, scale=10.0)

        def emit_rs(g):
            for s, t in enumerate(groups[g]):
                nc.vector.tensor_scalar(
                    scr[:, s * B : (s + 1) * B],
                    eg[g][:, s * B : (s + 1) * B],
                    1.0, None, OP.mult, OP.add,
                    accum_out=outt[:, t : t + 1],
                )

        # Pipeline: mm g0, mm g1 | exp g0.. with colsum matmuls for group g
        # emitted ahead of mm g+2 (which recycles g's PSUM banks anyway).
        emit_mms(0)
        emit_mms(1)
        emit_exp(0)
        emit_rs(0)
        for g in range(2, len(groups)):
            emit_mms(g)
            emit_exp(g - 1)
            emit_rs(g - 1)
            emit_cs(g - 1)
            if g == 3:
                # CE: predicts arrive behind ft; slot the exp mid-pipeline.
                ept = epool.tile([128, 4 * C], bf16, tag="ept")
                nc.scalar.activation(ept[:], predt[:], AF.Exp, bias=0.0, scale=1.0)
                for cchunk in range(4):
                    nc.vector.tensor_scalar(
                        scrp[:],
                        ept[:, cchunk * C : (cchunk + 1) * C],
                        1.0, None, OP.mult, OP.add,
                        accum_out=outt[:, NT + cchunk : NT + cchunk + 1],
                    )
        gl = len(groups) - 1
        emit_exp(gl)
        emit_rs(gl)
        emit_cs(gl)

        nc.scalar.copy(cs1s[:], cs1p[:])
        nc.sync.dma_start(cs1_d[:, :], cs1s[:])
        nc.sync.dma_start(out_d[:, :], outt[:])

    nc.compile()
    return nc


def _get_nc():
    if "nc" not in _CACHE:
        _CACHE["nc"] = _build_nc()
    return _CACHE["nc"]


def _prep_in_maps(predicts, labels, features):
    import ml_dtypes

    feats = np.ascontiguousarray(features, dtype=np.float32)
    pred = np.ascontiguousarray(predicts, dtype=np.float32)
    f8 = feats.reshape(B, FLIP, D).transpose(1, 0, 2)  # [8,512,128], f8[c]=feats[c::8]

    FT_COLS = NJ * B + 16
    oneh = np.zeros((D, 16), dtype=np.float32)
    for j in (1, 2, 3):
        oneh[:, 3 * (j - 1) + (j - 1)] = 1.0

    in_maps = []
    for a in range(FLIP):
        order = [(a + i) % FLIP for i in range(NJ)]
        fo = f8[order]  # [5, 512, 128]
        ft = np.empty((D, FT_COLS), dtype=np.float32)
        ft[:, : NJ * B] = fo.transpose(2, 0, 1).reshape(D, NJ * B)
        ft[:, NJ * B :] = oneh
        pb = pred[a * B : (a + 1) * B].reshape(4, 128, C).transpose(1, 0, 2)
        in_maps.append(
            {
                "ft": np.ascontiguousarray(ft).astype(ml_dtypes.bfloat16),
                "pred": np.ascontiguousarray(pb.reshape(128, 4 * C)),
            }
        )
    return in_maps


def _combine(outs, predicts, labels, features):
    """Host-side O(N*D) combine: reroute per-block sums between the
    ordered halves and apply the closed-form first-order series."""
    import ml_dtypes

    feats = np.asarray(features, dtype=np.float32)
    f8 = feats.reshape(B, FLIP, D).transpose(1, 0, 2).astype(np.float64)
    fb8 = f8.astype(ml_dtypes.bfloat16).astype(np.float64)  # device-side values

    # d_ab[p] = f_a[p].f_b[p] for every ordered pair (exact f32 features)
    dv = np.einsum("apd,bpd->abp", f8, f8)

    S1 = {}
    for c in range(FLIP):
        m = np.asarray(outs[c]["out"], np.float64)   # [128, 24]
        cs1 = np.asarray(outs[c]["cs1"], np.float64)  # [3, 512]
        for j in range(NJ):
            b = (c + j) % FLIP
            # col t=4j+r holds rowsums of rows 128r..128r+127
            S1[(c, b)] = m[:, 4 * j : 4 * j + 4].T.reshape(B)
        for j in (1, 2, 3):
            S1[((c + j) % FLIP, c)] = cs1[j - 1]

    # remove the raw diagonal exp from the own-block rowsums the same way
    # the device accumulated it: bf16(exp(10*||f_p||^2 in bf16 products))
    S10 = {}
    for c in range(FLIP):
        gpp = np.einsum("pd,pd->p", fb8[c], fb8[c])
        dg = np.exp(10.0 * gpp).astype(np.float32)
        dg = dg.astype(ml_dtypes.bfloat16).astype(np.float64)
        S10[c] = S1[(c, c)] - dg

    nce = 0.0
    for a in range(FLIP):
        for b in range(FLIP):
            d = dv[a, b]
            if a == b:
                N1 = 2.0 * S10[a]
                Dv = N1 + E10
                half = 10.0 * d - np.log(Dv) - N1 / Dv
                nce += 2.0 * half.sum()
            else:
                N1 = S10[a] + S1[(a, b)]
                half = (
                    10.0 * d
                    - np.log(N1)
                    - 1.0
                    - np.log1p(-np.exp(10.0 * d) / N1)
                )
                nce += half.sum()

    # CE: device exp-sums + host label gather
    pred = np.asarray(predicts, dtype=np.float64)
    lab = np.asarray(labels).astype(np.int64)
    xl = pred[np.arange(N), lab]
    ce = -xl.sum()
    for c in range(FLIP):
        se = np.asarray(outs[c]["out"], np.float64)[:, NT : NT + 4]  # [128,4]
        # se[p, cc] = sum_k exp(pred[c*512 + cc*128 + p, k])
        ce += np.log(se).T.reshape(B).sum()

    val = ALPHA * (-(nce) / 1024.0) + ce / N
    return np.array(val, dtype=np.float32)


def _run_hw(in_maps, trace=False):
    from concourse.bass_utils import run_bass_kernel_spmd

    nc = _get_nc()
    return run_bass_kernel_spmd(nc, in_maps, core_ids=list(range(FLIP)), trace=trace)


def kernel(predicts, labels, features, indexs=None, **_):
    in_maps = _prep_in_maps(predicts, labels, features)
    res = _run_hw(in_maps)
    return _combine(res.results, predicts, labels, features)


def kernel_sim(predicts, labels, features, indexs=None, **_):
    """CoreSim (CPU simulator) path for fast correctness iteration."""
    from concourse.bass_interp import CoreSim

    nc = _get_nc()
    in_maps = _prep_in_maps(predicts, labels, features)
    outs = []
    for a in range(FLIP):
        sim = CoreSim(nc, trace=False)
        for k, v in in_maps[a].items():
            sim.tensor(k)[:] = v
        sim.simulate()
        outs.append({k: np.array(sim.tensor(k)) for k in ("out", "cs1")})
    return _combine(outs, predicts, labels, features)


# revision 11
# speedup vs baseline: 1.7182x; 1.0539x over previous
"""Trainium2 Bass kernel for nn_BatchFlipLoss (NCE batch-flip loss + CE loss).

Math reformulation (validated ~1e-7 vs the jax reference in f64; the
first-order series below adds ~9e-5, vs a 2e-2 gate):

The reference sums BatchCriterion over 36 flip-class pairs (i,j), j>=i.
For pair (i,j) with x = [f_i; f_j] (f_c = features[c::8], L2-normalized,
B=512 rows each), T=0.1, the loss decomposes over ordered halves (a,b).
With E_ab = exp(10*G_ab), G_ab = f_a@f_b.T, S_ab = rowsum(E_ab),
d_ab[p] = f_a[p].f_b[p]:

  D_ab = S0_aa + S_ab      (S0_aa: diag-removed; (a,a): D = 2*S0_aa+e^10)
  half = 10*d - ln(D) - N1/D - ln(1 - exp(10 d)/D),  N1 = S0_aa + S_ab
  (a,a) pair = 2*(10*d - lnD - 2*S0_aa/D)
  ln(1-x) ~ -x only (the x^2/2 tail is ~9e-5 relative after scaling).

Work assignment: 36 unordered blocks over 8 cores = 4.5 each. Core c
computes blocks (c, c+j) j=0..3 in full (16 [128,512] Gram tiles) plus
HALF of its distance-4 block {c, c+4}: cores 0-3 take columns 0:256 of
E(f_c rows x f_{c+4} cols), cores 4-7 take rows 256:512 of the mirror
E(f_c rows x f_{c-4} cols) — identical instruction stream, different
host-packed lhsT/rhs inputs (four [128,256] matmuls each).

Device pipeline per core (SPMD, inputs rotated so own class is block 0):
  - Gram matmuls write 1-3 tiles into multi-bank PSUM tiles; ONE ACT exp
    per group ([128,512..1536]) converts to bf16 E in SBUF.
  - per-tile rowsums: DVE tensor_scalar(+accum_out) on bf16 E (4x DVE
    mode, accum free) -> out[:, t].
  - colsums for j in {1,2,3} (the partner core's rowsums): PE matmuls
    with one-hot lhsT accumulate into one [3,512] PSUM tile; the d4
    half-block colsums accumulate into a separate [2,256] PSUM tile.
  - CE: one ACT exp over [128,1600] predicts + DVE accum per 400-chunk.
  - diag of block (c,c) is NOT zeroed on device: the host subtracts
    bf16(exp(10*||f_p||^2_bf16)) from the raw diag rowsums.
The host does only O(N*D)/O(N) work: input layout, d_ab products, the
CE label gather, and the closed-form scalar combine.
"""

from contextlib import ExitStack

import numpy as np

FLIP = 8
B = 512
D = 128
C = 400
N = 4096
ALPHA = 0.03
E10 = float(np.exp(np.float32(10.0)))

# ftp column layout (bf16): 4 full blocks, d4 lhsT chunks, d4 rhs halves,
# one-hot columns for the j-colsums (3x3) and d4-colsums (2x2).
BLK = 4 * B            # 0:2048   blocks j=0..3
LHS4 = BLK             # 2048:2560
RHS4 = LHS4 + B        # 2560:3072
OHJ = RHS4 + B         # 3072:3081
OH4 = OHJ + 9          # 3081:3085
FT_COLS = 3088

# slot order: s0=(j0,r0) | s1..s12 = j1r0..j3r3 | s13,s14 = d4 halves |
# s15..s17 = (j0,r1..r3).  outt col t for slot rowsums (d4 slots use two
# cols each: 13,14 and 15,16), CE sums in cols 20:24.
GROUPS = [[0], [1, 2, 3], [4, 5, 6], [7, 8, 9], [10, 11, 12],
          [13, 14, 15], [16], [17]]
J0_COL = {0: 0, 1: 17, 2: 18, 3: 19}

_CACHE = {}


def _build_nc():
    import concourse.tile as tile
    from concourse import bacc, mybir

    f32 = mybir.dt.float32
    bf16 = mybir.dt.bfloat16
    AF = mybir.ActivationFunctionType
    OP = mybir.AluOpType

    nc = bacc.Bacc("TRN2", target_bir_lowering=False, debug=False)

    ft_d = nc.dram_tensor("ft", [D, FT_COLS], bf16, kind="ExternalInput")
    pred_d = nc.dram_tensor("pred", [128, 4 * C], f32, kind="ExternalInput")
    out_d = nc.dram_tensor("out", [128, 24], f32, kind="ExternalOutput")
    cs1_d = nc.dram_tensor("cs1", [5, B], f32, kind="ExternalOutput")

    def slot_info(s):
        """-> (kind, j, r) with kind in {'full','d4'}"""
        if s == 0:
            return ("full", 0, 0)
        if 1 <= s <= 12:
            return ("full", 1 + (s - 1) // 4, (s - 1) % 4)
        if s in (13, 14):
            return ("d4", None, s - 13)
        return ("full", 0, s - 14)

    with tile.TileContext(nc) as tc, ExitStack() as ctx:
        const = ctx.enter_context(tc.tile_pool(name="const", bufs=1))
        gpool = ctx.enter_context(tc.tile_pool(name="gp", bufs=2, space="PSUM"))
        cpool = ctx.enter_context(tc.tile_pool(name="cp", bufs=1, space="PSUM"))
        c4pool = ctx.enter_context(tc.tile_pool(name="c4", bufs=1, space="PSUM"))
        epool = ctx.enter_context(tc.tile_pool(name="ep", bufs=3))
        small = ctx.enter_context(tc.tile_pool(name="sm", bufs=1))

        ftt = const.tile([D, FT_COLS], bf16)
        predt = const.tile([128, 4 * C], f32)
        outt = small.tile([128, 24], f32)
        scr = small.tile([128, 3 * B], bf16)
        scrp = small.tile([128, C], bf16)
        cs1s = small.tile([3, B], f32)
        cs4s = small.tile([2, 256], f32)
        nc.sync.dma_start(ftt[:, 0:B], ft_d[:, 0:B])
        nc.sync.dma_start(ftt[:, B : 2 * B], ft_d[:, B : 2 * B])
        nc.sync.dma_start(ftt[:, 2 * B : FT_COLS], ft_d[:, 2 * B : FT_COLS])
        nc.sync.dma_start(predt[:], pred_d[:, :])

        cs1p = cpool.tile([3, B], f32)
        cs4p = c4pool.tile([2, 256], f32)

        pg = {}
        eg = {}

        def emit_mms(g):
            slots = GROUPS[g]
            pgt = gpool.tile([128, len(slots) * B], f32, tag="pg")
            pg[g] = pgt
            for i, s in enumerate(slots):
                kind, j, r = slot_info(s)
                if kind == "full":
                    nc.tensor.matmul(
                        pgt[:, i * B : (i + 1) * B],
                        ftt[:, r * 128 : (r + 1) * 128],
                        ftt[:, j * B : (j + 1) * B],
                        start=True,
                        stop=True,
                    )
                else:  # d4 half-slot: two [128,256] matmuls (quarters 2r,2r+1)
                    for h in range(2):
                        q = 2 * r + h
                        nc.tensor.matmul(
                            pgt[:, i * B + h * 256 : i * B + (h + 1) * 256],
                            ftt[:, LHS4 + q * 128 : LHS4 + (q + 1) * 128],
                            ftt[:, RHS4 + (q // 2) * 256 : RHS4 + (q // 2 + 1) * 256],
                            start=True,
                            stop=True,
                        )

        def emit_exp(g):
            egt = epool.tile([128, len(GROUPS[g]) * B], bf16, tag="eg")
            eg[g] = egt
            nc.scalar.activation(egt[:], pg[g][:], AF.Exp, bias=0.0, scale=10.0)

        def emit_rs(g):
            for i, s in enumerate(GROUPS[g]):
                kind, j, r = slot_info(s)
                if kind == "full":
                    col = J0_COL[r] if j == 0 else s
                    nc.vector.tensor_scalar(
                        scr[:, i * B : (i + 1) * B],
                        eg[g][:, i * B : (i + 1) * B],
                        1.0, None, OP.mult, OP.add,
                        accum_out=outt[:, col : col + 1],
                    )
                else:
                    for h in range(2):
                        col = 13 + 2 * r + h
                        nc.vector.tensor_scalar(
                            scr[:, i * B + h * 256 : i * B + (h + 1) * 256],
                            eg[g][:, i * B + h * 256 : i * B + (h + 1) * 256],
                            1.0, None, OP.mult, OP.add,
                            accum_out=outt[:, col : col + 1],
                        )

        def emit_cs(g):
            for i, s in enumerate(GROUPS[g]):
                kind, j, r = slot_info(s)
                if kind == "full":
                    if not (1 <= j <= 3):
                        continue
                    oh = OHJ + 3 * (j - 1)
                    nc.tensor.matmul(
                        cs1p[:],
                        ftt[:, oh : oh + 3],
                        eg[g][:, i * B : (i + 1) * B],
                        start=(s == 1),
                        stop=(s == 12),
                    )
                else:
                    for h in range(2):
                        q = 2 * r + h
                        oh = OH4 + 2 * (q // 2)
                        nc.tensor.matmul(
                            cs4p[:],
                            ftt[:, oh : oh + 2],
                            eg[g][:, i * B + h * 256 : i * B + (h + 1) * 256],
                            start=(q == 0),
                            stop=(q == 3),
                        )

        emit_mms(0)
        emit_mms(1)
        emit_exp(0)
        emit_rs(0)
        for g in range(2, len(GROUPS)):
            emit_mms(g)
            emit_exp(g - 1)
            emit_rs(g - 1)
            emit_cs(g - 1)
            if g == 5:
                # CE: predicts arrive behind ft; slot the exp mid-pipeline.
                ept = epool.tile([128, 4 * C], bf16, tag="ept")
                nc.scalar.activation(ept[:], predt[:], AF.Exp, bias=0.0, scale=1.0)
                for cchunk in range(4):
                    nc.vector.tensor_scalar(
                        scrp[:],
                        ept[:, cchunk * C : (cchunk + 1) * C],
                        1.0, None, OP.mult, OP.add,
                        accum_out=outt[:, 20 + cchunk : 21 + cchunk],
                    )
            if g == 7:
                # all colsum matmuls done (last is in emit_cs(5), d4 quarters)
                nc.vector.tensor_copy(cs1s[:], cs1p[:])
                nc.vector.tensor_copy(cs4s[:], cs4p[:])
                nc.gpsimd.dma_start(cs1_d[0:3, :], cs1s[:])
                nc.gpsimd.dma_start(cs1_d[3:5, 0:256], cs4s[:])
        gl = len(GROUPS) - 1
        emit_exp(gl)
        emit_rs(gl)

        nc.sync.dma_start(out_d[:, :], outt[:])

    nc.compile()
    return nc


def _get_nc():
    if "nc" not in _CACHE:
        _CACHE["nc"] = _build_nc()
    return _CACHE["nc"]


def _prep_in_maps(predicts, labels, features):
    import ml_dtypes

    feats = np.ascontiguousarray(features, dtype=np.float32)
    pred = np.ascontiguousarray(predicts, dtype=np.float32)
    f8 = feats.reshape(B, FLIP, D).transpose(1, 0, 2)  # [8,512,128], f8[c]=feats[c::8]

    ohj = np.zeros((D, 9), dtype=np.float32)
    for j in (1, 2, 3):
        ohj[:, 3 * (j - 1) + (j - 1)] = 1.0
    oh4 = np.zeros((D, 4), dtype=np.float32)
    oh4[:, 0] = 1.0  # quarters 0,1 -> cs4 row 0
    oh4[:, 3] = 1.0  # quarters 2,3 -> cs4 row 1

    in_maps = []
    for a in range(FLIP):
        order = [(a + i) % FLIP for i in range(4)]
        fo = f8[order]                       # [4, 512, 128] blocks j=0..3
        fp = f8[(a + 4) % FLIP]              # d4 partner [512, 128]
        ft = np.zeros((D, FT_COLS), dtype=np.float32)
        ft[:, :BLK] = fo.transpose(2, 0, 1).reshape(D, BLK)
        if a < 4:
            # columns 0:256 of E(f_a rows x f_partner cols): lhsT chunks =
            # own rows 0..3, rhs halves both = partner[0:256]
            ft[:, LHS4:RHS4] = f8[a].T
            ft[:, RHS4 : RHS4 + 256] = fp[0:256].T
            ft[:, RHS4 + 256 : OHJ] = fp[0:256].T
        else:
            # rows 256:512 of E(f_a rows x f_partner cols), all 512 columns
            own = f8[a]
            ft[:, LHS4 : LHS4 + 128] = own[256:384].T
            ft[:, LHS4 + 128 : LHS4 + 256] = own[384:512].T
            ft[:, LHS4 + 256 : LHS4 + 384] = own[256:384].T
            ft[:, LHS4 + 384 : RHS4] = own[384:512].T
            ft[:, RHS4:OHJ] = fp.T
        ft[:, OHJ:OH4] = ohj
        ft[:, OH4 : OH4 + 4] = oh4
        pb = pred[a * B : (a + 1) * B].reshape(4, 128, C).transpose(1, 0, 2)
        in_maps.append(
            {
                "ft": np.ascontiguousarray(ft).astype(ml_dtypes.bfloat16),
                "pred": np.ascontiguousarray(pb.reshape(128, 4 * C)),
            }
        )
    return in_maps


def _combine(outs, predicts, labels, features):
    """Host-side O(N*D) combine: reroute per-block sums between the
    ordered halves and apply the closed-form first-order series."""
    import ml_dtypes

    feats = np.asarray(features, dtype=np.float32)
    f8 = feats.reshape(B, FLIP, D).transpose(1, 0, 2).astype(np.float64)
    fb8 = f8.astype(ml_dtypes.bfloat16).astype(np.float64)  # device-side values

    dv = np.einsum("apd,bpd->abp", f8, f8)

    S1 = {}
    m = {}
    cs = {}
    for c in range(FLIP):
        m[c] = np.asarray(outs[c]["out"], np.float64)   # [128, 24]
        cs[c] = np.asarray(outs[c]["cs1"], np.float64)  # [5, 512]
        for j in range(4):
            b = (c + j) % FLIP
            if j == 0:
                cols = [J0_COL[r] for r in range(4)]
            else:
                cols = [1 + 4 * (j - 1) + r for r in range(4)]
            S1[(c, b)] = m[c][:, cols].T.reshape(B)
        for j in (1, 2, 3):
            S1[((c + j) % FLIP, c)] = cs[c][j - 1]

    # distance-4 pairs {b, b+4}, b < 4: stitch the two half-blocks
    for b in range(4):
        bp = b + 4
        # rowsums of M = E(f_b rows x f_bp cols)
        partial = m[b][:, 13:17].T.reshape(B)          # cols 0:256, all rows
        compl_lo = cs[bp][3, 0:256]                    # rows' missing cols, q<256
        compl_hi = cs[bp][4, 0:256]                    # q in 256:512
        S1[(b, bp)] = partial + np.concatenate([compl_lo, compl_hi])
        # rowsums of M^T = E(f_bp rows x f_b cols)
        lo = cs[b][3, 0:256] + cs[b][4, 0:256]         # rows 0:256 of f_bp
        hi = np.empty(256)
        hi[0:128] = m[bp][:, 13] + m[bp][:, 15]        # rows 256:384
        hi[128:256] = m[bp][:, 14] + m[bp][:, 16]      # rows 384:512
        S1[(bp, b)] = np.concatenate([lo, hi])

    # remove the raw diagonal exp from the own-block rowsums the same way
    # the device accumulated it: bf16(exp(10*||f_p||^2 in bf16 products))
    S10 = {}
    for c in range(FLIP):
        gpp = np.einsum("pd,pd->p", fb8[c], fb8[c])
        dg = np.exp(10.0 * gpp).astype(np.float32)
        dg = dg.astype(ml_dtypes.bfloat16).astype(np.float64)
        S10[c] = S1[(c, c)] - dg

    nce = 0.0
    for a in range(FLIP):
        for b in range(FLIP):
            d = dv[a, b]
            if a == b:
                N1 = 2.0 * S10[a]
                Dv = N1 + E10
                half = 10.0 * d - np.log(Dv) - N1 / Dv
                nce += 2.0 * half.sum()
            else:
                N1 = S10[a] + S1[(a, b)]
                half = (
                    10.0 * d
                    - np.log(N1)
                    - 1.0
                    - np.log1p(-np.exp(10.0 * d) / N1)
                )
                nce += half.sum()

    # CE: device exp-sums + host label gather
    pred = np.asarray(predicts, dtype=np.float64)
    lab = np.asarray(labels).astype(np.int64)
    xl = pred[np.arange(N), lab]
    ce = -xl.sum()
    for c in range(FLIP):
        se = m[c][:, 20:24]  # se[p, cc] = sum_k exp(pred[c*512+cc*128+p, k])
        ce += np.log(se).T.reshape(B).sum()

    val = ALPHA * (-(nce) / 1024.0) + ce / N
    return np.array(val, dtype=np.float32)


def _run_hw(in_maps, trace=False):
    from concourse.bass_utils import run_bass_kernel_spmd

    nc = _get_nc()
    return run_bass_kernel_spmd(nc, in_maps, core_ids=list(range(FLIP)), trace=trace)


def kernel(predicts, labels, features, indexs=None, **_):
    in_maps = _prep_in_maps(predicts, labels, features)
    res = _run_hw(in_maps)
    return _combine(res.results, predicts, labels, features)


def kernel_sim(predicts, labels, features, indexs=None, **_):
    """CoreSim (CPU simulator) path for fast correctness iteration."""
    from concourse.bass_interp import CoreSim

    nc = _get_nc()
    in_maps = _prep_in_maps(predicts, labels, features)
    outs = []
    for a in range(FLIP):
        sim = CoreSim(nc, trace=False)
        for k, v in in_maps[a].items():
            sim.tensor(k)[:] = v
        sim.simulate()
        outs.append({k: np.array(sim.tensor(k)) for k in ("out", "cs1")})
    return _combine(outs, predicts, labels, features)


# revision 13
# speedup vs baseline: 1.7251x; 1.0040x over previous
"""Trainium2 Bass kernel for nn_BatchFlipLoss (NCE batch-flip loss + CE loss).

Math reformulation (validated ~1e-7 vs the jax reference in f64; the
first-order series below adds ~9e-5, vs a 2e-2 gate):

The reference sums BatchCriterion over 36 flip-class pairs (i,j), j>=i.
For pair (i,j) with x = [f_i; f_j] (f_c = features[c::8], L2-normalized,
B=512 rows each), T=0.1, the loss decomposes over ordered halves (a,b).
With E_ab = exp(10*G_ab), G_ab = f_a@f_b.T, S_ab = rowsum(E_ab),
d_ab[p] = f_a[p].f_b[p]:

  D_ab = S0_aa + S_ab      (S0_aa: diag-removed; (a,a): D = 2*S0_aa+e^10)
  half = 10*d - ln(D) - N1/D - ln(1 - exp(10 d)/D),  N1 = S0_aa + S_ab
  (a,a) pair = 2*(10*d - lnD - 2*S0_aa/D)
  ln(1-x) ~ -x only (the x^2/2 tail is ~9e-5 relative after scaling).

Work assignment: 36 unordered blocks over 8 cores = 4.5 each. Core c
computes blocks (c, c+j) j=0..3 in full (16 [128,512] Gram tiles) plus
HALF of its distance-4 block {c, c+4}: cores 0-3 take columns 0:256 of
E(f_c rows x f_{c+4} cols), cores 4-7 take rows 256:512 of the mirror
E(f_c rows x f_{c-4} cols) — identical instruction stream, different
host-packed lhsT/rhs inputs (four [128,256] matmuls each).

Device pipeline per core (SPMD, inputs rotated so own class is block 0):
  - Gram matmuls write 1-3 tiles into multi-bank PSUM tiles; ONE ACT exp
    per group ([128,512..1536]) converts to bf16 E in SBUF.
  - per-tile rowsums: DVE tensor_scalar(+accum_out) on bf16 E (4x DVE
    mode, accum free) -> out[:, t].
  - colsums for j in {1,2,3} (the partner core's rowsums): PE matmuls
    with one-hot lhsT accumulate into one [3,512] PSUM tile; the d4
    half-block colsums accumulate into a separate [2,256] PSUM tile.
  - CE: one ACT exp over [128,1600] predicts + DVE accum per 400-chunk.
  - diag of block (c,c) is NOT zeroed on device: the host subtracts
    bf16(exp(10*||f_p||^2_bf16)) from the raw diag rowsums.
The host does only O(N*D)/O(N) work: input layout, d_ab products, the
CE label gather, and the closed-form scalar combine.
"""

from contextlib import ExitStack

import numpy as np

FLIP = 8
B = 512
D = 128
C = 400
N = 4096
ALPHA = 0.03
E10 = float(np.exp(np.float32(10.0)))

# ftp column layout (bf16): 4 full blocks, d4 lhsT chunks, d4 rhs halves,
# one-hot columns for the j-colsums (3x3) and d4-colsums (2x2).
BLK = 4 * B            # 0:2048   blocks j=0..3
LHS4 = BLK             # 2048:2560
RHS4 = LHS4 + B        # 2560:3072
OHJ = RHS4 + B         # 3072:3081
OH4 = OHJ + 9          # 3081:3085
FT_COLS = 3088

# slot ids: s0=(j0,r0) | s1..s12 = j1r0..j3r3 | s13,s14 = d4 halves |
# s15..s17 = (j0,r1..r3).  outt col t for slot rowsums (d4 slots use two
# cols each: 13,14 and 15,16), CE sums in cols 20:24.
# Group order is tuned for the DMA arrival schedule and the two pipeline
# tails: early groups need only feature block 0/1, the d4 slots wait for
# the third ft DMA, the j-colsum chain ends one group before the last so
# its staging+DMA overlaps the final exp, and the last group is a diag
# tile (no colsums) so only its rowsum gates the final output DMA.
GROUPS = [[0], [15, 16, 1], [2, 3, 4], [13, 14, 5], [6, 7, 8],
          [9, 10, 11], [12], [17]]
J0_COL = {0: 0, 1: 17, 2: 18, 3: 19}

_CACHE = {}


def _build_nc():
    import concourse.tile as tile
    from concourse import bacc, mybir

    f32 = mybir.dt.float32
    bf16 = mybir.dt.bfloat16
    AF = mybir.ActivationFunctionType
    OP = mybir.AluOpType

    nc = bacc.Bacc("TRN2", target_bir_lowering=False, debug=False)

    ft_d = nc.dram_tensor("ft", [D, FT_COLS], bf16, kind="ExternalInput")
    pred_d = nc.dram_tensor("pred", [128, 4 * C], f32, kind="ExternalInput")
    out_d = nc.dram_tensor("out", [128, 24], f32, kind="ExternalOutput")
    cs1_d = nc.dram_tensor("cs1", [5, B], f32, kind="ExternalOutput")

    def slot_info(s):
        """-> (kind, j, r) with kind in {'full','d4'}"""
        if s == 0:
            return ("full", 0, 0)
        if 1 <= s <= 12:
            return ("full", 1 + (s - 1) // 4, (s - 1) % 4)
        if s in (13, 14):
            return ("d4", None, s - 13)
        return ("full", 0, s - 14)

    with tile.TileContext(nc) as tc, ExitStack() as ctx:
        const = ctx.enter_context(tc.tile_pool(name="const", bufs=1))
        gpool = ctx.enter_context(tc.tile_pool(name="gp", bufs=2, space="PSUM"))
        cpool = ctx.enter_context(tc.tile_pool(name="cp", bufs=1, space="PSUM"))
        c4pool = ctx.enter_context(tc.tile_pool(name="c4", bufs=1, space="PSUM"))
        epool = ctx.enter_context(tc.tile_pool(name="ep", bufs=3))
        small = ctx.enter_context(tc.tile_pool(name="sm", bufs=1))

        ftt = const.tile([D, FT_COLS], bf16)
        predt = const.tile([128, 4 * C], f32)
        outt = small.tile([128, 24], f32)
        scr = small.tile([128, 3 * B], bf16)
        scrp = small.tile([128, C], bf16)
        cs1s = small.tile([3, B], f32)
        cs4s = small.tile([2, 256], f32)
        nc.sync.dma_start(ftt[:, 0:B], ft_d[:, 0:B])
        nc.sync.dma_start(ftt[:, B : 2 * B], ft_d[:, B : 2 * B])
        nc.sync.dma_start(ftt[:, 2 * B : FT_COLS], ft_d[:, 2 * B : FT_COLS])
        nc.sync.dma_start(predt[:], pred_d[:, :])

        cs1p = cpool.tile([3, B], f32)
        cs4p = c4pool.tile([2, 256], f32)

        pg = {}
        eg = {}

        def emit_mms(g):
            slots = GROUPS[g]
            pgt = gpool.tile([128, len(slots) * B], f32, tag="pg")
            pg[g] = pgt
            for i, s in enumerate(slots):
                kind, j, r = slot_info(s)
                if kind == "full":
                    nc.tensor.matmul(
                        pgt[:, i * B : (i + 1) * B],
                        ftt[:, r * 128 : (r + 1) * 128],
                        ftt[:, j * B : (j + 1) * B],
                        start=True,
                        stop=True,
                    )
                else:  # d4 half-slot: two [128,256] matmuls (quarters 2r,2r+1)
                    for h in range(2):
                        q = 2 * r + h
                        nc.tensor.matmul(
                            pgt[:, i * B + h * 256 : i * B + (h + 1) * 256],
                            ftt[:, LHS4 + q * 128 : LHS4 + (q + 1) * 128],
                            ftt[:, RHS4 + (q // 2) * 256 : RHS4 + (q // 2 + 1) * 256],
                            start=True,
                            stop=True,
                        )

        def emit_exp(g):
            egt = epool.tile([128, len(GROUPS[g]) * B], bf16, tag="eg")
            eg[g] = egt
            nc.scalar.activation(egt[:], pg[g][:], AF.Exp, bias=0.0, scale=10.0)

        def emit_rs(g):
            for i, s in enumerate(GROUPS[g]):
                kind, j, r = slot_info(s)
                if kind == "full":
                    col = J0_COL[r] if j == 0 else s
                    nc.vector.tensor_scalar(
                        scr[:, i * B : (i + 1) * B],
                        eg[g][:, i * B : (i + 1) * B],
                        1.0, None, OP.mult, OP.add,
                        accum_out=outt[:, col : col + 1],
                    )
                else:
                    for h in range(2):
                        col = 13 + 2 * r + h
                        nc.vector.tensor_scalar(
                            scr[:, i * B + h * 256 : i * B + (h + 1) * 256],
                            eg[g][:, i * B + h * 256 : i * B + (h + 1) * 256],
                            1.0, None, OP.mult, OP.add,
                            accum_out=outt[:, col : col + 1],
                        )

        def emit_cs(g):
            for i, s in enumerate(GROUPS[g]):
                kind, j, r = slot_info(s)
                if kind == "full":
                    if not (1 <= j <= 3):
                        continue
                    oh = OHJ + 3 * (j - 1)
                    nc.tensor.matmul(
                        cs1p[:],
                        ftt[:, oh : oh + 3],
                        eg[g][:, i * B : (i + 1) * B],
                        start=(s == 1),
                        stop=(s == 12),
                    )
                else:
                    for h in range(2):
                        q = 2 * r + h
                        oh = OH4 + 2 * (q // 2)
                        nc.tensor.matmul(
                            cs4p[:],
                            ftt[:, oh : oh + 2],
                            eg[g][:, i * B + h * 256 : i * B + (h + 1) * 256],
                            start=(q == 0),
                            stop=(q == 3),
                        )

        emit_mms(0)
        emit_mms(1)
        emit_exp(0)
        emit_rs(0)
        for g in range(2, len(GROUPS)):
            emit_mms(g)
            emit_exp(g - 1)
            emit_rs(g - 1)
            emit_cs(g - 1)
            if g == 4:
                # CE: predicts arrive behind ft; slot the exp mid-pipeline.
                ept = epool.tile([128, 4 * C], bf16, tag="ept")
                nc.scalar.activation(ept[:], predt[:], AF.Exp, bias=0.0, scale=1.0)
                for cchunk in range(4):
                    nc.vector.tensor_scalar(
                        scrp[:],
                        ept[:, cchunk * C : (cchunk + 1) * C],
                        1.0, None, OP.mult, OP.add,
                        accum_out=outt[:, 20 + cchunk : 21 + cchunk],
                    )
            if g == 7:
                # all colsum matmuls done (last is in emit_cs(5), d4 quarters)
                nc.vector.tensor_copy(cs1s[:], cs1p[:])
                nc.vector.tensor_copy(cs4s[:], cs4p[:])
                nc.gpsimd.dma_start(cs1_d[0:3, :], cs1s[:])
                nc.gpsimd.dma_start(cs1_d[3:5, 0:256], cs4s[:])
        gl = len(GROUPS) - 1
        emit_exp(gl)
        emit_rs(gl)

        nc.sync.dma_start(out_d[:, :], outt[:])

    nc.compile()
    return nc


def _get_nc():
    if "nc" not in _CACHE:
        _CACHE["nc"] = _build_nc()
    return _CACHE["nc"]


def _prep_in_maps(predicts, labels, features):
    import ml_dtypes

    feats = np.ascontiguousarray(features, dtype=np.float32)
    pred = np.ascontiguousarray(predicts, dtype=np.float32)
    f8 = feats.reshape(B, FLIP, D).transpose(1, 0, 2)  # [8,512,128], f8[c]=feats[c::8]

    ohj = np.zeros((D, 9), dtype=np.float32)
    for j in (1, 2, 3):
        ohj[:, 3 * (j - 1) + (j - 1)] = 1.0
    oh4 = np.zeros((D, 4), dtype=np.float32)
    oh4[:, 0] = 1.0  # quarters 0,1 -> cs4 row 0
    oh4[:, 3] = 1.0  # quarters 2,3 -> cs4 row 1

    in_maps = []
    for a in range(FLIP):
        order = [(a + i) % FLIP for i in range(4)]
        fo = f8[order]                       # [4, 512, 128] blocks j=0..3
        fp = f8[(a + 4) % FLIP]              # d4 partner [512, 128]
        ft = np.zeros((D, FT_COLS), dtype=np.float32)
        ft[:, :BLK] = fo.transpose(2, 0, 1).reshape(D, BLK)
        if a < 4:
            # columns 0:256 of E(f_a rows x f_partner cols): lhsT chunks =
            # own rows 0..3, rhs halves both = partner[0:256]
            ft[:, LHS4:RHS4] = f8[a].T
            ft[:, RHS4 : RHS4 + 256] = fp[0:256].T
            ft[:, RHS4 + 256 : OHJ] = fp[0:256].T
        else:
            # rows 256:512 of E(f_a rows x f_partner cols), all 512 columns
            own = f8[a]
            ft[:, LHS4 : LHS4 + 128] = own[256:384].T
            ft[:, LHS4 + 128 : LHS4 + 256] = own[384:512].T
            ft[:, LHS4 + 256 : LHS4 + 384] = own[256:384].T
            ft[:, LHS4 + 384 : RHS4] = own[384:512].T
            ft[:, RHS4:OHJ] = fp.T
        ft[:, OHJ:OH4] = ohj
        ft[:, OH4 : OH4 + 4] = oh4
        pb = pred[a * B : (a + 1) * B].reshape(4, 128, C).transpose(1, 0, 2)
        in_maps.append(
            {
                "ft": np.ascontiguousarray(ft).astype(ml_dtypes.bfloat16),
                "pred": np.ascontiguousarray(pb.reshape(128, 4 * C)),
            }
        )
    return in_maps


def _combine(outs, predicts, labels, features):
    """Host-side O(N*D) combine: reroute per-block sums between the
    ordered halves and apply the closed-form first-order series."""
    import ml_dtypes

    feats = np.asarray(features, dtype=np.float32)
    f8 = feats.reshape(B, FLIP, D).transpose(1, 0, 2).astype(np.float64)
    fb8 = f8.astype(ml_dtypes.bfloat16).astype(np.float64)  # device-side values

    dv = np.einsum("apd,bpd->abp", f8, f8)

    S1 = {}
    m = {}
    cs = {}
    for c in range(FLIP):
        m[c] = np.asarray(outs[c]["out"], np.float64)   # [128, 24]
        cs[c] = np.asarray(outs[c]["cs1"], np.float64)  # [5, 512]
        for j in range(4):
            b = (c + j) % FLIP
            if j == 0:
                cols = [J0_COL[r] for r in range(4)]
            else:
                cols = [1 + 4 * (j - 1) + r for r in range(4)]
            S1[(c, b)] = m[c][:, cols].T.reshape(B)
        for j in (1, 2, 3):
            S1[((c + j) % FLIP, c)] = cs[c][j - 1]

    # distance-4 pairs {b, b+4}, b < 4: stitch the two half-blocks
    for b in range(4):
        bp = b + 4
        # rowsums of M = E(f_b rows x f_bp cols)
        partial = m[b][:, 13:17].T.reshape(B)          # cols 0:256, all rows
        compl_lo = cs[bp][3, 0:256]                    # rows' missing cols, q<256
        compl_hi = cs[bp][4, 0:256]                    # q in 256:512
        S1[(b, bp)] = partial + np.concatenate([compl_lo, compl_hi])
        # rowsums of M^T = E(f_bp rows x f_b cols)
        lo = cs[b][3, 0:256] + cs[b][4, 0:256]         # rows 0:256 of f_bp
        hi = np.empty(256)
        hi[0:128] = m[bp][:, 13] + m[bp][:, 15]        # rows 256:384
        hi[128:256] = m[bp][:, 14] + m[bp][:, 16]      # rows 384:512
        S1[(bp, b)] = np.concatenate([lo, hi])

    # remove the raw diagonal exp from the own-block rowsums the same way
    # the device accumulated it: bf16(exp(10*||f_p||^2 in bf16 products))
    S10 = {}
    for c in range(FLIP):
        gpp = np.einsum("pd,pd->p", fb8[c], fb8[c])
        dg = np.exp(10.0 * gpp).astype(np.float32)
        dg = dg.astype(ml_dtypes.bfloat16).astype(np.float64)
        S10[c] = S1[(c, c)] - dg

    nce = 0.0
    for a in range(FLIP):
        for b in range(FLIP):
            d = dv[a, b]
            if a == b:
                N1 = 2.0 * S10[a]
                Dv = N1 + E10
                half = 10.0 * d - np.log(Dv) - N1 / Dv
                nce += 2.0 * half.sum()
            else:
                N1 = S10[a] + S1[(a, b)]
                half = (
                    10.0 * d
                    - np.log(N1)
                    - 1.0
                    - np.log1p(-np.exp(10.0 * d) / N1)
                )
                nce += half.sum()

    # CE: device exp-sums + host label gather
    pred = np.asarray(predicts, dtype=np.float64)
    lab = np.asarray(labels).astype(np.int64)
    xl = pred[np.arange(N), lab]
    ce = -xl.sum()
    for c in range(FLIP):
        se = m[c][:, 20:24]  # se[p, cc] = sum_k exp(pred[c*512+cc*128+p, k])
        ce += np.log(se).T.reshape(B).sum()

    val = ALPHA * (-(nce) / 1024.0) + ce / N
    return np.array(val, dtype=np.float32)


def _run_hw(in_maps, trace=False):
    from concourse.bass_utils import run_bass_kernel_spmd

    nc = _get_nc()
    return run_bass_kernel_spmd(nc, in_maps, core_ids=list(range(FLIP)), trace=trace)


def kernel(predicts, labels, features, indexs=None, **_):
    in_maps = _prep_in_maps(predicts, labels, features)
    res = _run_hw(in_maps)
    return _combine(res.results, predicts, labels, features)


def kernel_sim(predicts, labels, features, indexs=None, **_):
    """CoreSim (CPU simulator) path for fast correctness iteration."""
    from concourse.bass_interp import CoreSim

    nc = _get_nc()
    in_maps = _prep_in_maps(predicts, labels, features)
    outs = []
    for a in range(FLIP):
        sim = CoreSim(nc, trace=False)
        for k, v in in_maps[a].items():
            sim.tensor(k)[:] = v
        sim.simulate()
        outs.append({k: np.array(sim.tensor(k)) for k in ("out", "cs1")})
    return _combine(outs, predicts, labels, features)
